# revision 40
# baseline (speedup 1.0000x reference)
"""Trainium2 Bass kernel for nn_IterativeStructureRefiner (v5, bracket form).

Math (validated vs reference in numpy: fp32 2.9e-7, bf16-quantized 3.6e-3):
  s' = 0.75*s + g .* [ Qd + A.*S1 + B.*S2 + C.*S3 ]
    g  = 1 - unc                      (gate, applied once per iter)
    Qd = 0.25*(box3x3(s) - s)         (PE: V36@sL + V36@sR + T8@s)
    S1 = Bm@hx, S2 = T0@cs + H0@hx, S3 = A0v@csR - A0v@csL
         (cs = continuity*s, hx = csL+csR)
    A  = q*oxx, B = q - A, C = q*oxy,  q = 0.25/(den+eps)  (iteration-
    invariant; den = same stencils on continuity, using oyy = 1-oxx).

State substitution y_t = s_t/0.75^t makes the update y' = y + g'.*bracket
(g' = (1-unc)/0.75), so each iteration ends in a plain tensor_add; the
0.75^6 is folded into the final fp32 store-copy on the Scalar engine.

Engine split per strip-iteration (128x1024) - GpSimd is deliberately
IDLE: it shares an SBUF port with the Vector engine and 2-port DVE
tensor_tensor ops starve it (measured 2.4us ops inflating to 8-11us,
stalling the PE chain and oscillating HAM):
  Vector: cs, hx, fused [A|B].*[S1|S2] (one 2048-wide 2x op), C-product,
          3 adds, gate product, y-add (9 tensor_tensor ops/iter)
  Scalar: 2 super-drains per iter (PSUM [S1|S2] and [S3|Qd] pairs as
          single 2048-wide fp32->bf16 ACTIVATE copies)
  Tensor: 16 matmuls (512-col fp32-PSUM chunks), 7 banded stationaries
  The pre-phase den product is similarly fused: [oxy|oxx].*[S3c|d12c]
  with d12c parked in the d3q super-tile's free upper half.

Geometry: 9 full-width row strips [128 x 1024], 6-row shrink halo (116
valid rows/strip). 3 strips in flight (one wave), 3 waves; next wave's
DMA loads (at it==1) and full pre-chain (at it==3) are interleaved into
the current wave's iterations so wave boundaries stay busy; wave 0 hoists
it=0 produce/matmuls ahead of the den-chain. PSUM: 2 shared [128,2048]
fp32 super-tiles (8 banks), drained promptly so slots rotate.

Sharding: pure data-parallel, one batch image per NeuronCore (B=8).
"""

import numpy as np

H = W = 1024
TILE_W = 1040          # tile col t <-> image col t - 8 ; pads [0:8) [1032:1040)
W0, W1 = 8, 1032
PW = W1 - W0           # 1024
ROWS_OUT = 116
NUM_ITERS = 6
N_SLOTS = 3

_CACHE = {}


def _build_bass():
    import concourse.bacc as bacc
    import concourse.mybir as mybir
    from concourse.tile import TileContext

    fp32 = mybir.dt.float32
    bf16 = mybir.dt.bfloat16
    Alu = mybir.AluOpType
    Act = mybir.ActivationFunctionType

    nc = bacc.Bacc("TRN2", debug=False)

    cen_d = nc.dram_tensor("center", [H, W], bf16, kind="ExternalInput")
    con_d = nc.dram_tensor("continuity", [H, W], bf16, kind="ExternalInput")
    ori_d = nc.dram_tensor("orientation", [2, H, W], bf16, kind="ExternalInput")
    unc_d = nc.dram_tensor("uncertainty", [H, W], bf16, kind="ExternalInput")
    out_d = nc.dram_tensor("out", [H, W], fp32, kind="ExternalOutput")

    # Stationaries (lhsT convention: out[i] = sum_k lhsT[k,i] * x[k]).
    k = np.arange(128)
    I128 = np.eye(128, dtype=np.float32)
    T0m_ = (np.abs(k[:, None] - k[None, :]) == 1).astype(np.float32)
    Bm = I128 + 0.5 * T0m_
    H0 = 0.5 * T0m_
    A0v = ((k[:, None] == k[None, :] + 1).astype(np.float32)
           - (k[:, None] == k[None, :] - 1).astype(np.float32))
    V36 = (I128 + T0m_) * (1.0 / 36.0)
    T8 = (T0m_ - 8.0 * I128) * (1.0 / 36.0)
    ST = [Bm, T0m_, H0, A0v, -A0v, V36, T8]
    st_drams = [nc.inline_tensor(m, name=f"st_{i}") for i, m in enumerate(ST)]

    row_panels = []
    for r0 in range(0, H, ROWS_OUT):
        row_panels.append((r0, min(r0 + ROWS_OUT, H)))
    assert len(row_panels) == 9

    CH = ((0, 512), (512, 1024))   # psum col j <-> tile col j + W0
    WSL = slice(W0, W1)

    with TileContext(nc) as tc:
        with (
            tc.tile_pool(name="consts", bufs=1) as kpool,
            tc.tile_pool(name="cin", bufs=2) as c2pool,
            tc.tile_pool(name="inp", bufs=2) as ipool,
            tc.tile_pool(name="coef", bufs=1) as fpool,
            tc.tile_pool(name="state", bufs=1) as spool,
            tc.tile_pool(name="outp", bufs=1) as opool,
            tc.tile_pool(name="psum", bufs=1, space="PSUM") as qpool,
        ):
            st_b = []
            s_ab = {}

            def stage_consts():
                for i, d in enumerate(st_drams):
                    tf = kpool.tile([128, 128], fp32, tag="stf")
                    nc.sync.dma_start(out=tf[:], in_=d[:, :])
                    tb = kpool.tile([128, 128], bf16, tag=f"st{i}")
                    nc.scalar.copy(tb[:], tf[:])
                    st_b.append(tb)
                # persistent s ping-pong tiles per slot; pads zeroed once
                for sl in range(N_SLOTS):
                    pair = []
                    for nm in ("sa", "sb"):
                        t = kpool.tile([128, TILE_W], bf16, tag=f"{nm}{sl}")
                        nc.vector.memset(t[:, 0:W0], 0.0)
                        nc.vector.memset(t[:, W1:TILE_W], 0.0)
                        pair.append(t)
                    s_ab[sl] = pair

            class Panel:
                pass

            def make_panel(r0, r1, sl, wp):
                P = Panel()
                P.r0, P.r1, P.sl, P.wp = r0, r1, sl, wp
                P.row_lo = max(r0 - 6, 0)
                P.row_hi = min(r0 + 122, H)
                P.p_lo = P.row_lo - (r0 - 6)
                P.p_hi = P.row_hi - (r0 - 6)
                return P

            def emit_load(P):
                sl = P.sl

                def row_pads(t, lo_val, hi_val, win):
                    if P.p_lo > 0:
                        nc.gpsimd.memset(t[0:P.p_lo, win], lo_val)
                    if P.p_hi < 128:
                        al = (P.p_hi // 32) * 32
                        nc.gpsimd.memset(t[al:128, win], hi_val)

                # continuity: padded 1040-wide tile (feeds stencils)
                ct = c2pool.tile([128, TILE_W], bf16, tag=f"c{sl}")
                nc.vector.memset(ct[:, 0:W0], 0.0)
                nc.vector.memset(ct[:, W1:TILE_W], 0.0)
                row_pads(ct, 0.0, 0.0, WSL)
                nc.sync.dma_start(out=ct[P.p_lo:P.p_hi, WSL],
                                  in_=con_d[P.row_lo:P.row_hi, :])
                P.c_t = ct
                # center: padded 1040-wide (initial s)
                st = ipool.tile([128, TILE_W], bf16, tag=f"s0{sl}")
                nc.vector.memset(st[:, 0:W0], 0.0)
                nc.vector.memset(st[:, W1:TILE_W], 0.0)
                row_pads(st, 0.0, 0.0, WSL)
                nc.sync.dma_start(out=st[P.p_lo:P.p_hi, WSL],
                                  in_=cen_d[P.row_lo:P.row_hi, :])
                P.s0_t = st
                P.s_cur = st
                # ox, oy: 1024-wide, zero row pads (NaN hygiene)
                full = slice(0, PW)
                for tag, src in ((f"ox{sl}", ori_d[0]), (f"oy{sl}", ori_d[1])):
                    t = ipool.tile([128, PW], bf16, tag=tag)
                    row_pads(t, 0.0, 0.0, full)
                    nc.sync.dma_start(out=t[P.p_lo:P.p_hi, :],
                                      in_=src[P.row_lo:P.row_hi, :])
                    setattr(P, "t_" + tag[:2], t)
                # uncertainty: pads = 1.0 so g = 1-unc = 0 outside image
                t = ipool.tile([128, PW], bf16, tag=f"un{sl}")
                row_pads(t, 1.0, 1.0, full)
                nc.sync.dma_start(out=t[P.p_lo:P.p_hi, :],
                                  in_=unc_d[P.row_lo:P.row_hi, :])
                P.t_un = t

            def emit_mm_group(q, passes):
                """passes = [(stationary, moving_window_fn)]; accumulate into
                PSUM region q (column range of a super-tile)."""
                n = len(passes)
                for pi, (stat, win) in enumerate(passes):
                    for lo, hi in CH:
                        nc.tensor.matmul(q[:, lo:hi], stat[:], win(lo, hi),
                                         start=(pi == 0), stop=(pi == n - 1))

            def emit_pre_a(P):
                """Maps + den stencils on PE + drains (cheap engines)."""
                sl = P.sl
                # paired [oxy|oxx] map tile enables a fused den-product below
                oo = spool.tile([128, 2 * PW], bf16, tag=f"oo{sl}")
                oxy, oxx = oo[:, 0:PW], oo[:, PW:2 * PW]
                g = fpool.tile([128, PW], bf16, tag=f"g{sl}p{P.wp}")
                nc.scalar.activation(oxx, P.t_ox[:], Act.Square)
                nc.vector.tensor_mul(out=oxy, in0=P.t_ox[:], in1=P.t_oy[:])
                # state substitution y_t = s_t / 0.75^t turns the update into
                # y' = y + g'.*bracket with g' = (1-unc)/0.75; the 0.75^6 is
                # applied once in the final fp32 store-copy.
                nc.scalar.activation(g[:], P.t_un[:], Act.Copy,
                                     bias=1.0 / 0.75, scale=-1.0 / 0.75)
                P.oo, P.oxx, P.oxy, P.g = oo, oxx, oxy, g

                c = P.c_t
                qA = qpool.tile([128, 2 * PW], fp32, tag="qA")
                qB = qpool.tile([128, 2 * PW], fp32, tag="qB")
                d12 = spool.tile([128, 2 * PW], bf16, tag=f"d13{sl}")
                d3q = spool.tile([128, 2 * PW], bf16, tag=f"d2q{sl}")
                emit_mm_group(qA[:, 0:PW], [
                    (tBm, lambda lo, hi: c[:, W0 - 1 + lo:W0 - 1 + hi]),
                    (tBm, lambda lo, hi: c[:, W0 + 1 + lo:W0 + 1 + hi]),
                ])
                emit_mm_group(qA[:, PW:2 * PW], [
                    (tT0, lambda lo, hi: c[:, W0 + lo:W0 + hi]),
                    (tH0, lambda lo, hi: c[:, W0 - 1 + lo:W0 - 1 + hi]),
                    (tH0, lambda lo, hi: c[:, W0 + 1 + lo:W0 + 1 + hi]),
                ])
                nc.scalar.copy(d12[:], qA[:])
                emit_mm_group(qB[:, 0:PW], [
                    (tA0v, lambda lo, hi: c[:, W0 + 1 + lo:W0 + 1 + hi]),
                    (tA0vm, lambda lo, hi: c[:, W0 - 1 + lo:W0 - 1 + hi]),
                ])
                nc.scalar.copy(d3q[:, 0:PW], qB[:, 0:PW])
                P.d12t, P.d3q = d12, d3q

            def emit_pre_b(P):
                """den chain + coefficient maps (mostly DVE)."""
                sl = P.sl
                S1b, S2b = P.d12t[:, 0:PW], P.d12t[:, PW:2 * PW]
                S3b = P.d3q[:, 0:PW]
                p2b = spool.tile([128, 2 * PW], bf16, tag=f"p2b{sl}")
                t3 = spool.tile([128, PW], bf16, tag=f"t3{sl}")
                t4 = spool.tile([128, PW], bf16, tag=f"t4{sl}")
                # d12c parked in d3q's free upper half -> [S3c|d12c] pairs
                # with [oxy|oxx] for one fused 2048-wide den product
                d12c = P.d3q[:, PW:2 * PW]
                nc.vector.tensor_sub(out=d12c, in0=S1b, in1=S2b)
                nc.vector.tensor_mul(out=p2b[:], in0=P.oo[:], in1=P.d3q[:])
                nc.vector.tensor_add(out=t3[:], in0=p2b[:, 0:PW],
                                     in1=p2b[:, PW:2 * PW])
                nc.vector.tensor_add(out=t4[:], in0=t3[:], in1=S2b)
                e4 = opool.tile([128, PW], fp32, tag=f"of{sl}")
                nc.vector.tensor_scalar(out=e4[:], in0=t4[:], scalar1=4.0,
                                        scalar2=4e-6, op0=Alu.mult,
                                        op1=Alu.add)
                nc.vector.reciprocal_approx_fast(out=e4[:], in_=e4[:])
                qrb = p2b[:, PW:2 * PW]
                nc.scalar.copy(qrb, e4[:])
                AB = fpool.tile([128, 2 * PW], bf16, tag=f"AB{sl}p{P.wp}")
                C = fpool.tile([128, PW], bf16, tag=f"C{sl}p{P.wp}")
                nc.vector.tensor_mul(out=AB[:, 0:PW], in0=qrb, in1=P.oxx[:])
                nc.vector.tensor_mul(out=C[:], in0=qrb, in1=P.oxy[:])
                nc.vector.tensor_sub(out=AB[:, PW:2 * PW], in0=qrb,
                                     in1=AB[:, 0:PW])
                P.AB, P.C = AB, C

            def emit_produce(P, it):
                sl = P.sl
                cs = spool.tile([128, TILE_W], bf16, tag=f"cs{sl}")
                nc.vector.tensor_mul(out=cs[:], in0=P.c_t[:], in1=P.s_cur[:])
                hs = spool.tile([128, TILE_W - 2], bf16, tag=f"hs{sl}")
                nc.vector.tensor_add(out=hs[:], in0=P.s_cur[:, 0:TILE_W - 2],
                                     in1=P.s_cur[:, 2:TILE_W])
                P.cs, P.hs = cs, hs

            def emit_mm(P, it):
                sl = P.sl
                cs, s = P.cs, P.s_cur
                qA = qpool.tile([128, 2 * PW], fp32, tag="qA")
                qB = qpool.tile([128, 2 * PW], fp32, tag="qB")
                d12 = spool.tile([128, 2 * PW], bf16, tag=f"d13{sl}")
                d3q = spool.tile([128, 2 * PW], bf16, tag=f"d2q{sl}")
                emit_mm_group(qA[:, 0:PW], [
                    (tBm, lambda lo, hi: cs[:, W0 - 1 + lo:W0 - 1 + hi]),
                    (tBm, lambda lo, hi: cs[:, W0 + 1 + lo:W0 + 1 + hi]),
                ])
                emit_mm_group(qA[:, PW:2 * PW], [
                    (tT0, lambda lo, hi: cs[:, W0 + lo:W0 + hi]),
                    (tH0, lambda lo, hi: cs[:, W0 - 1 + lo:W0 - 1 + hi]),
                    (tH0, lambda lo, hi: cs[:, W0 + 1 + lo:W0 + 1 + hi]),
                ])
                nc.scalar.copy(d12[:], qA[:])
                emit_mm_group(qB[:, 0:PW], [
                    (tA0v, lambda lo, hi: cs[:, W0 + 1 + lo:W0 + 1 + hi]),
                    (tA0vm, lambda lo, hi: cs[:, W0 - 1 + lo:W0 - 1 + hi]),
                ])
                hs = P.hs
                emit_mm_group(qB[:, PW:2 * PW], [
                    (tV36, lambda lo, hi: hs[:, W0 - 1 + lo:W0 - 1 + hi]),
                    (tT8, lambda lo, hi: s[:, W0 + lo:W0 + hi]),
                ])
                nc.scalar.copy(d3q[:], qB[:])
                P.d12t, P.d3q = d12, d3q

            def emit_tail(P, it):
                sl = P.sl
                S3b, Qdb = P.d3q[:, 0:PW], P.d3q[:, PW:2 * PW]
                # fused [A|B] .* [S1|S2] in one 2048-wide 2x op
                p2b = spool.tile([128, 2 * PW], bf16, tag=f"p2b{sl}")
                t3 = spool.tile([128, PW], bf16, tag=f"t3{sl}")
                t4 = spool.tile([128, PW], bf16, tag=f"t4{sl}")
                nc.vector.tensor_mul(out=p2b[:], in0=P.AB[:], in1=P.d12t[:])
                nc.vector.tensor_add(out=t3[:], in0=p2b[:, 0:PW],
                                     in1=p2b[:, PW:2 * PW])
                nc.vector.tensor_mul(out=t4[:], in0=P.C[:], in1=S3b)
                nc.vector.tensor_add(out=t4[:], in0=t4[:], in1=Qdb)
                nc.vector.tensor_add(out=t3[:], in0=t3[:], in1=t4[:])
                pg = spool.tile([128, PW], bf16, tag=f"t4{sl}")
                nc.vector.tensor_mul(out=pg[:], in0=P.g[:], in1=t3[:])
                last = it == NUM_ITERS - 1
                if last:
                    yb = spool.tile([128, PW], bf16, tag=f"t3{sl}")
                    nc.vector.tensor_add(out=yb[:], in0=P.s_cur[:, WSL],
                                         in1=pg[:])
                    s_nxt = opool.tile([128, PW], fp32, tag=f"of{sl}")
                    nc.scalar.activation(s_nxt[:], yb[:], Act.Copy,
                                         bias=0.0, scale=0.75 ** NUM_ITERS)
                else:
                    s_nxt = s_ab[P.sl][it % 2]
                    nc.vector.tensor_add(out=s_nxt[:, WSL],
                                         in0=P.s_cur[:, WSL], in1=pg[:])
                P.s_cur = s_nxt

            def emit_store(P):
                nrows = P.r1 - P.r0
                nc.sync.dma_start(out=out_d[P.r0:P.r1, :],
                                  in_=P.s_cur[6:6 + nrows, :])

            waves = []
            for wi in range(0, len(row_panels), N_SLOTS):
                waves.append([make_panel(*row_panels[wi + j], j,
                                         (wi // N_SLOTS) % 2)
                              for j in range(min(N_SLOTS, len(row_panels) - wi))])
            stage_consts()
            tBm, tT0, tH0, tA0v, tA0vm, tV36, tT8 = st_b
            for P in waves[0]:
                emit_load(P)
            for w, wave in enumerate(waves):
                if w == 0:
                    # it=0 produce/mm depend only on loads, not on the
                    # coefficient pre-chain: start the PE early and let the
                    # den-chain overlap it.
                    for P in wave:
                        emit_pre_a(P)
                    for P in wave:
                        emit_produce(P, 0)
                    for P in wave:
                        emit_pre_b(P)
                    for P in wave:
                        emit_mm(P, 0)
                for it in range(NUM_ITERS):
                    if w == 0 and it == 0:
                        for P in wave:
                            emit_tail(P, it)
                        continue
                    for P in wave:
                        emit_produce(P, it)
                    for P in wave:
                        emit_mm(P, it)
                    for P in wave:
                        emit_tail(P, it)
                    # interleave next wave's loads + pre into this wave's
                    # iterations so engines stay fed across wave boundaries
                    if w + 1 < len(waves):
                        nxt = waves[w + 1]
                        if it == 1:
                            for P in nxt:
                                emit_load(P)
                        elif it == 3:
                            for P in nxt:
                                emit_pre_a(P)
                                emit_pre_b(P)
                for P in wave:
                    emit_store(P)

    nc.finalize()
    return nc


def kernel(center, continuity, orientation, uncertainty):
    from concourse.bass_utils import run_bass_kernel_spmd

    if "nc" not in _CACHE:
        _CACHE["nc"] = _build_bass()
    nc = _CACHE["nc"]

    import ml_dtypes
    bf = ml_dtypes.bfloat16
    B = center.shape[0]
    in_maps = []
    for b in range(B):
        in_maps.append({
            "center": np.ascontiguousarray(center[b, 0]).astype(bf),
            "continuity": np.ascontiguousarray(continuity[b, 0]).astype(bf),
            "orientation": np.ascontiguousarray(orientation[b]).astype(bf),
            "uncertainty": np.ascontiguousarray(uncertainty[b, 0]).astype(bf),
        })
    res = run_bass_kernel_spmd(nc, in_maps, core_ids=list(range(B)))
    out = np.stack([r["out"] for r in res.results])[:, None]
    return out.astype(np.float32)


# revision 42
# speedup vs baseline: 1.0254x; 1.0254x over previous
"""Trainium2 Bass kernel for nn_IterativeStructureRefiner (v5, bracket form).

Math (validated vs reference in numpy: fp32 2.9e-7, bf16-quantized 3.6e-3):
  s' = 0.75*s + g .* [ Qd + A.*S1 + B.*S2 + C.*S3 ]
    g  = 1 - unc                      (gate, applied once per iter)
    Qd = 0.25*(box3x3(s) - s)         (PE: V36@sL + V36@sR + T8@s)
    S1 = Bm@hx, S2 = T0@cs + H0@hx, S3 = A0v@csR - A0v@csL
         (cs = continuity*s, hx = csL+csR)
    A  = q*oxx, B = q - A, C = q*oxy,  q = 0.25/(den+eps)  (iteration-
    invariant; den = same stencils on continuity, using oyy = 1-oxx).

State substitution y_t = s_t/0.75^t makes the update y' = y + g'.*bracket
(g' = (1-unc)/0.75), so each iteration ends in a plain tensor_add; the
0.75^6 is folded into the final fp32 store-copy on the Scalar engine.

Engine split per strip-iteration (128x1024) - GpSimd is deliberately
IDLE: it shares an SBUF port with the Vector engine and 2-port DVE
tensor_tensor ops starve it (measured 2.4us ops inflating to 8-11us,
stalling the PE chain and oscillating HAM):
  Vector: cs, fused [A|B].*[S1|S2] (one 2048-wide 2x op), C-product,
          3 adds, gate product, y-add (8 tensor_tensor ops/iter; hx is
          absorbed into the PE via Bm/H0 passes on csL and csR)
  Scalar: 2 super-drains per iter (PSUM [S1|S2] and [S3|Qd] pairs as
          single 2048-wide fp32->bf16 ACTIVATE copies)
  Tensor: 20 matmuls (512-col fp32-PSUM chunks), 7 banded stationaries
  The pre-phase den product is similarly fused: [oxy|oxx].*[S3c|d12c]
  with d12c parked in the d3q super-tile's free upper half.

Geometry: 9 full-width row strips [128 x 1024], 6-row shrink halo (116
valid rows/strip). 3 strips in flight (one wave), 3 waves; next wave's
DMA loads (at it==1) and full pre-chain (at it==3) are interleaved into
the current wave's iterations so wave boundaries stay busy; wave 0 hoists
it=0 produce/matmuls ahead of the den-chain. PSUM: 2 shared [128,2048]
fp32 super-tiles (8 banks), drained promptly so slots rotate.

Sharding: pure data-parallel, one batch image per NeuronCore (B=8).
"""

import numpy as np

H = W = 1024
TILE_W = 1040          # tile col t <-> image col t - 8 ; pads [0:8) [1032:1040)
W0, W1 = 8, 1032
PW = W1 - W0           # 1024
ROWS_OUT = 116
NUM_ITERS = 6
N_SLOTS = 3

_CACHE = {}


def _build_bass():
    import concourse.bacc as bacc
    import concourse.mybir as mybir
    from concourse.tile import TileContext

    fp32 = mybir.dt.float32
    bf16 = mybir.dt.bfloat16
    Alu = mybir.AluOpType
    Act = mybir.ActivationFunctionType

    nc = bacc.Bacc("TRN2", debug=False)

    cen_d = nc.dram_tensor("center", [H, W], bf16, kind="ExternalInput")
    con_d = nc.dram_tensor("continuity", [H, W], bf16, kind="ExternalInput")
    ori_d = nc.dram_tensor("orientation", [2, H, W], bf16, kind="ExternalInput")
    unc_d = nc.dram_tensor("uncertainty", [H, W], bf16, kind="ExternalInput")
    out_d = nc.dram_tensor("out", [H, W], fp32, kind="ExternalOutput")

    # Stationaries (lhsT convention: out[i] = sum_k lhsT[k,i] * x[k]).
    k = np.arange(128)
    I128 = np.eye(128, dtype=np.float32)
    T0m_ = (np.abs(k[:, None] - k[None, :]) == 1).astype(np.float32)
    Bm = I128 + 0.5 * T0m_
    H0 = 0.5 * T0m_
    A0v = ((k[:, None] == k[None, :] + 1).astype(np.float32)
           - (k[:, None] == k[None, :] - 1).astype(np.float32))
    V36 = (I128 + T0m_) * (1.0 / 36.0)
    T8 = (T0m_ - 8.0 * I128) * (1.0 / 36.0)
    ST = [Bm, T0m_, H0, A0v, -A0v, V36, T8]
    st_drams = [nc.inline_tensor(m, name=f"st_{i}") for i, m in enumerate(ST)]

    row_panels = []
    for r0 in range(0, H, ROWS_OUT):
        row_panels.append((r0, min(r0 + ROWS_OUT, H)))
    assert len(row_panels) == 9

    CH = ((0, 512), (512, 1024))   # psum col j <-> tile col j + W0
    WSL = slice(W0, W1)

    with TileContext(nc) as tc:
        with (
            tc.tile_pool(name="consts", bufs=1) as kpool,
            tc.tile_pool(name="cin", bufs=2) as c2pool,
            tc.tile_pool(name="inp", bufs=2) as ipool,
            tc.tile_pool(name="coef", bufs=1) as fpool,
            tc.tile_pool(name="state", bufs=1) as spool,
            tc.tile_pool(name="outp", bufs=1) as opool,
            tc.tile_pool(name="psum", bufs=1, space="PSUM") as qpool,
        ):
            st_b = []
            s_ab = {}

            def stage_consts():
                for i, d in enumerate(st_drams):
                    tf = kpool.tile([128, 128], fp32, tag="stf")
                    nc.sync.dma_start(out=tf[:], in_=d[:, :])
                    tb = kpool.tile([128, 128], bf16, tag=f"st{i}")
                    nc.scalar.copy(tb[:], tf[:])
                    st_b.append(tb)
                # persistent s ping-pong tiles per slot; pads zeroed once
                for sl in range(N_SLOTS):
                    pair = []
                    for nm in ("sa", "sb"):
                        t = kpool.tile([128, TILE_W], bf16, tag=f"{nm}{sl}")
                        nc.vector.memset(t[:, 0:W0], 0.0)
                        nc.vector.memset(t[:, W1:TILE_W], 0.0)
                        pair.append(t)
                    s_ab[sl] = pair

            class Panel:
                pass

            def make_panel(r0, r1, sl, wp):
                P = Panel()
                P.r0, P.r1, P.sl, P.wp = r0, r1, sl, wp
                P.row_lo = max(r0 - 6, 0)
                P.row_hi = min(r0 + 122, H)
                P.p_lo = P.row_lo - (r0 - 6)
                P.p_hi = P.row_hi - (r0 - 6)
                return P

            def emit_load(P):
                sl = P.sl

                def row_pads(t, lo_val, hi_val, win):
                    if P.p_lo > 0:
                        nc.gpsimd.memset(t[0:P.p_lo, win], lo_val)
                    if P.p_hi < 128:
                        al = (P.p_hi // 32) * 32
                        nc.gpsimd.memset(t[al:128, win], hi_val)

                # continuity: padded 1040-wide tile (feeds stencils)
                ct = c2pool.tile([128, TILE_W], bf16, tag=f"c{sl}")
                nc.vector.memset(ct[:, 0:W0], 0.0)
                nc.vector.memset(ct[:, W1:TILE_W], 0.0)
                row_pads(ct, 0.0, 0.0, WSL)
                nc.sync.dma_start(out=ct[P.p_lo:P.p_hi, WSL],
                                  in_=con_d[P.row_lo:P.row_hi, :])
                P.c_t = ct
                # center: padded 1040-wide (initial s)
                st = ipool.tile([128, TILE_W], bf16, tag=f"s0{sl}")
                nc.vector.memset(st[:, 0:W0], 0.0)
                nc.vector.memset(st[:, W1:TILE_W], 0.0)
                row_pads(st, 0.0, 0.0, WSL)
                nc.sync.dma_start(out=st[P.p_lo:P.p_hi, WSL],
                                  in_=cen_d[P.row_lo:P.row_hi, :])
                P.s0_t = st
                P.s_cur = st
                # ox, oy: 1024-wide, zero row pads (NaN hygiene)
                full = slice(0, PW)
                for tag, src in ((f"ox{sl}", ori_d[0]), (f"oy{sl}", ori_d[1])):
                    t = ipool.tile([128, PW], bf16, tag=tag)
                    row_pads(t, 0.0, 0.0, full)
                    nc.sync.dma_start(out=t[P.p_lo:P.p_hi, :],
                                      in_=src[P.row_lo:P.row_hi, :])
                    setattr(P, "t_" + tag[:2], t)
                # uncertainty: pads = 1.0 so g = 1-unc = 0 outside image
                t = ipool.tile([128, PW], bf16, tag=f"un{sl}")
                row_pads(t, 1.0, 1.0, full)
                nc.sync.dma_start(out=t[P.p_lo:P.p_hi, :],
                                  in_=unc_d[P.row_lo:P.row_hi, :])
                P.t_un = t

            def emit_mm_group(q, passes):
                """passes = [(stationary, moving_window_fn)]; accumulate into
                PSUM region q (column range of a super-tile)."""
                n = len(passes)
                for pi, (stat, win) in enumerate(passes):
                    for lo, hi in CH:
                        nc.tensor.matmul(q[:, lo:hi], stat[:], win(lo, hi),
                                         start=(pi == 0), stop=(pi == n - 1))

            def emit_pre_a(P):
                """Maps + den stencils on PE + drains (cheap engines)."""
                sl = P.sl
                # paired [oxy|oxx] map tile enables a fused den-product below
                oo = spool.tile([128, 2 * PW], bf16, tag=f"oo{sl}")
                oxy, oxx = oo[:, 0:PW], oo[:, PW:2 * PW]
                g = fpool.tile([128, PW], bf16, tag=f"g{sl}p{P.wp}")
                nc.scalar.activation(oxx, P.t_ox[:], Act.Square)
                nc.vector.tensor_mul(out=oxy, in0=P.t_ox[:], in1=P.t_oy[:])
                # state substitution y_t = s_t / 0.75^t turns the update into
                # y' = y + g'.*bracket with g' = (1-unc)/0.75; the 0.75^6 is
                # applied once in the final fp32 store-copy.
                nc.scalar.activation(g[:], P.t_un[:], Act.Copy,
                                     bias=1.0 / 0.75, scale=-1.0 / 0.75)
                P.oo, P.oxx, P.oxy, P.g = oo, oxx, oxy, g

                c = P.c_t
                qA = qpool.tile([128, 2 * PW], fp32, tag="qA")
                qB = qpool.tile([128, 2 * PW], fp32, tag="qB")
                d12 = spool.tile([128, 2 * PW], bf16, tag=f"d13{sl}")
                d3q = spool.tile([128, 2 * PW], bf16, tag=f"d2q{sl}")
                emit_mm_group(qA[:, 0:PW], [
                    (tBm, lambda lo, hi: c[:, W0 - 1 + lo:W0 - 1 + hi]),
                    (tBm, lambda lo, hi: c[:, W0 + 1 + lo:W0 + 1 + hi]),
                ])
                emit_mm_group(qA[:, PW:2 * PW], [
                    (tT0, lambda lo, hi: c[:, W0 + lo:W0 + hi]),
                    (tH0, lambda lo, hi: c[:, W0 - 1 + lo:W0 - 1 + hi]),
                    (tH0, lambda lo, hi: c[:, W0 + 1 + lo:W0 + 1 + hi]),
                ])
                nc.scalar.copy(d12[:], qA[:])
                emit_mm_group(qB[:, 0:PW], [
                    (tA0v, lambda lo, hi: c[:, W0 + 1 + lo:W0 + 1 + hi]),
                    (tA0vm, lambda lo, hi: c[:, W0 - 1 + lo:W0 - 1 + hi]),
                ])
                nc.scalar.copy(d3q[:, 0:PW], qB[:, 0:PW])
                P.d12t, P.d3q = d12, d3q

            def emit_pre_b(P):
                """den chain + coefficient maps (mostly DVE)."""
                sl = P.sl
                S1b, S2b = P.d12t[:, 0:PW], P.d12t[:, PW:2 * PW]
                S3b = P.d3q[:, 0:PW]
                p2b = spool.tile([128, 2 * PW], bf16, tag=f"p2b{sl}")
                t3 = spool.tile([128, PW], bf16, tag=f"t3{sl}")
                t4 = spool.tile([128, PW], bf16, tag=f"t4{sl}")
                # d12c parked in d3q's free upper half -> [S3c|d12c] pairs
                # with [oxy|oxx] for one fused 2048-wide den product
                d12c = P.d3q[:, PW:2 * PW]
                nc.vector.tensor_sub(out=d12c, in0=S1b, in1=S2b)
                nc.vector.tensor_mul(out=p2b[:], in0=P.oo[:], in1=P.d3q[:])
                nc.vector.tensor_add(out=t3[:], in0=p2b[:, 0:PW],
                                     in1=p2b[:, PW:2 * PW])
                nc.vector.tensor_add(out=t4[:], in0=t3[:], in1=S2b)
                e4 = opool.tile([128, PW], fp32, tag=f"of{sl}")
                nc.vector.tensor_scalar(out=e4[:], in0=t4[:], scalar1=4.0,
                                        scalar2=4e-6, op0=Alu.mult,
                                        op1=Alu.add)
                nc.vector.reciprocal_approx_fast(out=e4[:], in_=e4[:])
                qrb = p2b[:, PW:2 * PW]
                nc.scalar.copy(qrb, e4[:])
                AB = fpool.tile([128, 2 * PW], bf16, tag=f"AB{sl}p{P.wp}")
                C = fpool.tile([128, PW], bf16, tag=f"C{sl}p{P.wp}")
                nc.vector.tensor_mul(out=AB[:, 0:PW], in0=qrb, in1=P.oxx[:])
                nc.vector.tensor_mul(out=C[:], in0=qrb, in1=P.oxy[:])
                nc.vector.tensor_sub(out=AB[:, PW:2 * PW], in0=qrb,
                                     in1=AB[:, 0:PW])
                P.AB, P.C = AB, C

            def emit_produce(P, it):
                sl = P.sl
                cs = spool.tile([128, TILE_W], bf16, tag=f"cs{sl}")
                nc.vector.tensor_mul(out=cs[:], in0=P.c_t[:], in1=P.s_cur[:])
                P.cs = cs

            def emit_mm(P, it):
                sl = P.sl
                cs, s = P.cs, P.s_cur
                qA = qpool.tile([128, 2 * PW], fp32, tag="qA")
                qB = qpool.tile([128, 2 * PW], fp32, tag="qB")
                d12 = spool.tile([128, 2 * PW], bf16, tag=f"d13{sl}")
                d3q = spool.tile([128, 2 * PW], bf16, tag=f"d2q{sl}")
                emit_mm_group(qA[:, 0:PW], [
                    (tBm, lambda lo, hi: cs[:, W0 - 1 + lo:W0 - 1 + hi]),
                    (tBm, lambda lo, hi: cs[:, W0 + 1 + lo:W0 + 1 + hi]),
                ])
                emit_mm_group(qA[:, PW:2 * PW], [
                    (tT0, lambda lo, hi: cs[:, W0 + lo:W0 + hi]),
                    (tH0, lambda lo, hi: cs[:, W0 - 1 + lo:W0 - 1 + hi]),
                    (tH0, lambda lo, hi: cs[:, W0 + 1 + lo:W0 + 1 + hi]),
                ])
                nc.scalar.copy(d12[:], qA[:])
                emit_mm_group(qB[:, 0:PW], [
                    (tA0v, lambda lo, hi: cs[:, W0 + 1 + lo:W0 + 1 + hi]),
                    (tA0vm, lambda lo, hi: cs[:, W0 - 1 + lo:W0 - 1 + hi]),
                ])
                emit_mm_group(qB[:, PW:2 * PW], [
                    (tV36, lambda lo, hi: s[:, W0 - 1 + lo:W0 - 1 + hi]),
                    (tV36, lambda lo, hi: s[:, W0 + 1 + lo:W0 + 1 + hi]),
                    (tT8, lambda lo, hi: s[:, W0 + lo:W0 + hi]),
                ])
                nc.scalar.copy(d3q[:], qB[:])
                P.d12t, P.d3q = d12, d3q

            def emit_tail(P, it):
                sl = P.sl
                S3b, Qdb = P.d3q[:, 0:PW], P.d3q[:, PW:2 * PW]
                # fused [A|B] .* [S1|S2] in one 2048-wide 2x op
                p2b = spool.tile([128, 2 * PW], bf16, tag=f"p2b{sl}")
                t3 = spool.tile([128, PW], bf16, tag=f"t3{sl}")
                t4 = spool.tile([128, PW], bf16, tag=f"t4{sl}")
                nc.vector.tensor_mul(out=p2b[:], in0=P.AB[:], in1=P.d12t[:])
                nc.vector.tensor_add(out=t3[:], in0=p2b[:, 0:PW],
                                     in1=p2b[:, PW:2 * PW])
                nc.vector.tensor_mul(out=t4[:], in0=P.C[:], in1=S3b)
                nc.vector.tensor_add(out=t4[:], in0=t4[:], in1=Qdb)
                nc.vector.tensor_add(out=t3[:], in0=t3[:], in1=t4[:])
                pg = spool.tile([128, PW], bf16, tag=f"t4{sl}")
                nc.vector.tensor_mul(out=pg[:], in0=P.g[:], in1=t3[:])
                last = it == NUM_ITERS - 1
                if last:
                    yb = spool.tile([128, PW], bf16, tag=f"t3{sl}")
                    nc.vector.tensor_add(out=yb[:], in0=P.s_cur[:, WSL],
                                         in1=pg[:])
                    s_nxt = opool.tile([128, PW], fp32, tag=f"of{sl}")
                    nc.scalar.activation(s_nxt[:], yb[:], Act.Copy,
                                         bias=0.0, scale=0.75 ** NUM_ITERS)
                else:
                    s_nxt = s_ab[P.sl][it % 2]
                    nc.vector.tensor_add(out=s_nxt[:, WSL],
                                         in0=P.s_cur[:, WSL], in1=pg[:])
                P.s_cur = s_nxt

            def emit_store(P):
                nrows = P.r1 - P.r0
                nc.sync.dma_start(out=out_d[P.r0:P.r1, :],
                                  in_=P.s_cur[6:6 + nrows, :])

            waves = []
            for wi in range(0, len(row_panels), N_SLOTS):
                waves.append([make_panel(*row_panels[wi + j], j,
                                         (wi // N_SLOTS) % 2)
                              for j in range(min(N_SLOTS, len(row_panels) - wi))])
            stage_consts()
            tBm, tT0, tH0, tA0v, tA0vm, tV36, tT8 = st_b
            for P in waves[0]:
                emit_load(P)
            for w, wave in enumerate(waves):
                if w == 0:
                    # it=0 produce/mm depend only on loads, not on the
                    # coefficient pre-chain: start the PE early and let the
                    # den-chain overlap it.
                    for P in wave:
                        emit_pre_a(P)
                    for P in wave:
                        emit_produce(P, 0)
                    for P in wave:
                        emit_pre_b(P)
                    for P in wave:
                        emit_mm(P, 0)
                for it in range(NUM_ITERS):
                    if w == 0 and it == 0:
                        for P in wave:
                            emit_tail(P, it)
                        continue
                    for P in wave:
                        emit_produce(P, it)
                    for P in wave:
                        emit_mm(P, it)
                    for P in wave:
                        emit_tail(P, it)
                    # interleave next wave's loads + pre into this wave's
                    # iterations so engines stay fed across wave boundaries
                    if w + 1 < len(waves):
                        nxt = waves[w + 1]
                        if it == 1:
                            for P in nxt:
                                emit_load(P)
                        elif it == 3:
                            for P in nxt:
                                emit_pre_a(P)
                                emit_pre_b(P)
                for P in wave:
                    emit_store(P)

    nc.finalize()
    return nc


def kernel(center, continuity, orientation, uncertainty):
    from concourse.bass_utils import run_bass_kernel_spmd

    if "nc" not in _CACHE:
        _CACHE["nc"] = _build_bass()
    nc = _CACHE["nc"]

    import ml_dtypes
    bf = ml_dtypes.bfloat16
    B = center.shape[0]
    in_maps = []
    for b in range(B):
        in_maps.append({
            "center": np.ascontiguousarray(center[b, 0]).astype(bf),
            "continuity": np.ascontiguousarray(continuity[b, 0]).astype(bf),
            "orientation": np.ascontiguousarray(orientation[b]).astype(bf),
            "uncertainty": np.ascontiguousarray(uncertainty[b, 0]).astype(bf),
        })
    res = run_bass_kernel_spmd(nc, in_maps, core_ids=list(range(B)))
    out = np.stack([r["out"] for r in res.results])[:, None]
    return out.astype(np.float32)


# revision 43
# speedup vs baseline: 1.0256x; 1.0001x over previous
"""Trainium2 Bass kernel for nn_IterativeStructureRefiner (v5, bracket form).

Math (validated vs reference in numpy: fp32 2.9e-7, bf16-quantized 3.6e-3):
  s' = 0.75*s + g .* [ Qd + A.*S1 + B.*S2 + C.*S3 ]
    g  = 1 - unc                      (gate, applied once per iter)
    Qd = 0.25*(box3x3(s) - s)         (PE: V36@sL + V36@sR + T8@s)
    S1 = Bm@hx, S2 = T0@cs + H0@hx, S3 = A0v@csR - A0v@csL
         (cs = continuity*s, hx = csL+csR)
    A  = q*oxx, B = q - A, C = q*oxy,  q = 0.25/(den+eps)  (iteration-
    invariant; den = same stencils on continuity, using oyy = 1-oxx).

State substitution y_t = s_t/0.75^t makes the update y' = y + g'.*bracket
(g' = (1-unc)/0.75), so each iteration ends in a plain tensor_add; the
0.75^6 is folded into the final fp32 store-copy on the Scalar engine.

Engine split per strip-iteration (128x1024) - GpSimd is deliberately
IDLE: it shares an SBUF port with the Vector engine and 2-port DVE
tensor_tensor ops starve it (measured 2.4us ops inflating to 8-11us,
stalling the PE chain and oscillating HAM):
  Vector: cs, fused [A|B].*[S1|S2] (one 2048-wide 2x op), C-product,
          3 adds, gate product, y-add (8 tensor_tensor ops/iter; hx is
          absorbed into the PE via Bm/H0 passes on csL and csR)
  Scalar: 2 super-drains per iter (PSUM [S1|S2] and [S3|Qd] pairs as
          single 2048-wide fp32->bf16 ACTIVATE copies)
  Tensor: 20 matmuls (512-col fp32-PSUM chunks), 7 banded stationaries
  The pre-phase den product is similarly fused: [oxy|oxx].*[S3c|d12c]
  with d12c parked in the d3q super-tile's free upper half.

Geometry: 9 full-width row strips [128 x 1024], 6-row shrink halo (116
valid rows/strip). 3 strips in flight (one wave), 3 waves; next wave's
DMA loads (at it==1) and full pre-chain (at it==3) are interleaved into
the current wave's iterations so wave boundaries stay busy; wave 0 hoists
it=0 produce/matmuls ahead of the den-chain. PSUM: 2 shared [128,2048]
fp32 super-tiles (8 banks), drained promptly so slots rotate.

Sharding: pure data-parallel, one batch image per NeuronCore (B=8).
"""

import numpy as np

H = W = 1024
TILE_W = 1040          # tile col t <-> image col t - 8 ; pads [0:8) [1032:1040)
W0, W1 = 8, 1032
PW = W1 - W0           # 1024
ROWS_OUT = 116
NUM_ITERS = 6
N_SLOTS = 3

_CACHE = {}


def _build_bass():
    import concourse.bacc as bacc
    import concourse.mybir as mybir
    from concourse.tile import TileContext

    fp32 = mybir.dt.float32
    bf16 = mybir.dt.bfloat16
    Alu = mybir.AluOpType
    Act = mybir.ActivationFunctionType

    nc = bacc.Bacc("TRN2", debug=False)

    cen_d = nc.dram_tensor("center", [H, W], bf16, kind="ExternalInput")
    con_d = nc.dram_tensor("continuity", [H, W], bf16, kind="ExternalInput")
    ori_d = nc.dram_tensor("orientation", [2, H, W], bf16, kind="ExternalInput")
    unc_d = nc.dram_tensor("uncertainty", [H, W], bf16, kind="ExternalInput")
    out_d = nc.dram_tensor("out", [H, W], fp32, kind="ExternalOutput")

    # Stationaries (lhsT convention: out[i] = sum_k lhsT[k,i] * x[k]).
    k = np.arange(128)
    I128 = np.eye(128, dtype=np.float32)
    T0m_ = (np.abs(k[:, None] - k[None, :]) == 1).astype(np.float32)
    Bm = I128 + 0.5 * T0m_
    H0 = 0.5 * T0m_
    A0v = ((k[:, None] == k[None, :] + 1).astype(np.float32)
           - (k[:, None] == k[None, :] - 1).astype(np.float32))
    V36 = (I128 + T0m_) * (1.0 / 36.0)
    T8 = (T0m_ - 8.0 * I128) * (1.0 / 36.0)
    ST = [Bm, T0m_, H0, A0v, -A0v, V36, T8]
    st_drams = [nc.inline_tensor(m, name=f"st_{i}") for i, m in enumerate(ST)]

    row_panels = []
    for r0 in range(0, H, ROWS_OUT):
        row_panels.append((r0, min(r0 + ROWS_OUT, H)))
    assert len(row_panels) == 9

    CH = ((0, 512), (512, 1024))   # psum col j <-> tile col j + W0
    WSL = slice(W0, W1)

    with TileContext(nc) as tc:
        with (
            tc.tile_pool(name="consts", bufs=1) as kpool,
            tc.tile_pool(name="cin", bufs=2) as c2pool,
            tc.tile_pool(name="inp", bufs=2) as ipool,
            tc.tile_pool(name="coef", bufs=1) as fpool,
            tc.tile_pool(name="state", bufs=1) as spool,
            tc.tile_pool(name="outp", bufs=1) as opool,
            tc.tile_pool(name="psum", bufs=1, space="PSUM") as qpool,
        ):
            st_b = []
            s_ab = {}

            def stage_consts():
                for i, d in enumerate(st_drams):
                    tf = kpool.tile([128, 128], fp32, tag="stf")
                    nc.sync.dma_start(out=tf[:], in_=d[:, :])
                    tb = kpool.tile([128, 128], bf16, tag=f"st{i}")
                    nc.scalar.copy(tb[:], tf[:])
                    st_b.append(tb)
                # persistent s ping-pong tiles per slot; pads zeroed once
                for sl in range(N_SLOTS):
                    pair = []
                    for nm in ("sa", "sb"):
                        t = kpool.tile([128, TILE_W], bf16, tag=f"{nm}{sl}")
                        nc.vector.memset(t[:, 0:W0], 0.0)
                        nc.vector.memset(t[:, W1:TILE_W], 0.0)
                        pair.append(t)
                    s_ab[sl] = pair

            class Panel:
                pass

            def make_panel(r0, r1, sl, wp):
                P = Panel()
                P.r0, P.r1, P.sl, P.wp = r0, r1, sl, wp
                P.row_lo = max(r0 - 6, 0)
                P.row_hi = min(r0 + 122, H)
                P.p_lo = P.row_lo - (r0 - 6)
                P.p_hi = P.row_hi - (r0 - 6)
                return P

            def emit_load(P):
                sl = P.sl

                def row_pads(t, lo_val, hi_val, win):
                    if P.p_lo > 0:
                        nc.gpsimd.memset(t[0:P.p_lo, win], lo_val)
                    if P.p_hi < 128:
                        al = (P.p_hi // 32) * 32
                        nc.gpsimd.memset(t[al:128, win], hi_val)

                # continuity: padded 1040-wide tile (feeds stencils)
                ct = c2pool.tile([128, TILE_W], bf16, tag=f"c{sl}")
                nc.vector.memset(ct[:, 0:W0], 0.0)
                nc.vector.memset(ct[:, W1:TILE_W], 0.0)
                row_pads(ct, 0.0, 0.0, WSL)
                nc.sync.dma_start(out=ct[P.p_lo:P.p_hi, WSL],
                                  in_=con_d[P.row_lo:P.row_hi, :])
                P.c_t = ct
                # center: padded 1040-wide (initial s)
                st = ipool.tile([128, TILE_W], bf16, tag=f"s0{sl}")
                nc.vector.memset(st[:, 0:W0], 0.0)
                nc.vector.memset(st[:, W1:TILE_W], 0.0)
                row_pads(st, 0.0, 0.0, WSL)
                nc.sync.dma_start(out=st[P.p_lo:P.p_hi, WSL],
                                  in_=cen_d[P.row_lo:P.row_hi, :])
                P.s0_t = st
                P.s_cur = st
                # ox, oy: 1024-wide, zero row pads (NaN hygiene)
                full = slice(0, PW)
                for tag, src in ((f"ox{sl}", ori_d[0]), (f"oy{sl}", ori_d[1])):
                    t = ipool.tile([128, PW], bf16, tag=tag)
                    row_pads(t, 0.0, 0.0, full)
                    nc.sync.dma_start(out=t[P.p_lo:P.p_hi, :],
                                      in_=src[P.row_lo:P.row_hi, :])
                    setattr(P, "t_" + tag[:2], t)
                # uncertainty: pads = 1.0 so g = 1-unc = 0 outside image
                t = ipool.tile([128, PW], bf16, tag=f"un{sl}")
                row_pads(t, 1.0, 1.0, full)
                nc.sync.dma_start(out=t[P.p_lo:P.p_hi, :],
                                  in_=unc_d[P.row_lo:P.row_hi, :])
                P.t_un = t

            def emit_mm_group(q, passes):
                """passes = [(stationary, moving_window_fn)]; accumulate into
                PSUM region q (column range of a super-tile)."""
                n = len(passes)
                for pi, (stat, win) in enumerate(passes):
                    for lo, hi in CH:
                        nc.tensor.matmul(q[:, lo:hi], stat[:], win(lo, hi),
                                         start=(pi == 0), stop=(pi == n - 1))

            def emit_pre_a(P):
                """Maps + den stencils on PE + drains (cheap engines)."""
                sl = P.sl
                # paired [oxy|oxx] map tile enables a fused den-product below
                oo = spool.tile([128, 2 * PW], bf16, tag=f"oo{sl}")
                oxy, oxx = oo[:, 0:PW], oo[:, PW:2 * PW]
                g = fpool.tile([128, PW], bf16, tag=f"g{sl}p{P.wp}")
                nc.scalar.activation(oxx, P.t_ox[:], Act.Square)
                nc.vector.tensor_mul(out=oxy, in0=P.t_ox[:], in1=P.t_oy[:])
                # state substitution y_t = s_t / 0.75^t turns the update into
                # y' = y + g'.*bracket with g' = (1-unc)/0.75; the 0.75^6 is
                # applied once in the final fp32 store-copy.
                nc.scalar.activation(g[:], P.t_un[:], Act.Copy,
                                     bias=1.0 / 0.75, scale=-1.0 / 0.75)
                P.oo, P.oxx, P.oxy, P.g = oo, oxx, oxy, g

                c = P.c_t
                qA = qpool.tile([128, 2 * PW], fp32, tag="qA")
                qB = qpool.tile([128, 2 * PW], fp32, tag="qB")
                d12 = spool.tile([128, 2 * PW], bf16, tag=f"d13{sl}")
                d3q = spool.tile([128, 2 * PW], bf16, tag=f"d2q{sl}")
                emit_mm_group(qA[:, 0:PW], [
                    (tBm, lambda lo, hi: c[:, W0 - 1 + lo:W0 - 1 + hi]),
                    (tBm, lambda lo, hi: c[:, W0 + 1 + lo:W0 + 1 + hi]),
                ])
                emit_mm_group(qA[:, PW:2 * PW], [
                    (tT0, lambda lo, hi: c[:, W0 + lo:W0 + hi]),
                    (tH0, lambda lo, hi: c[:, W0 - 1 + lo:W0 - 1 + hi]),
                    (tH0, lambda lo, hi: c[:, W0 + 1 + lo:W0 + 1 + hi]),
                ])
                nc.scalar.copy(d12[:], qA[:])
                emit_mm_group(qB[:, 0:PW], [
                    (tA0v, lambda lo, hi: c[:, W0 + 1 + lo:W0 + 1 + hi]),
                    (tA0vm, lambda lo, hi: c[:, W0 - 1 + lo:W0 - 1 + hi]),
                ])
                nc.scalar.copy(d3q[:, 0:PW], qB[:, 0:PW])
                P.d12t, P.d3q = d12, d3q

            def emit_pre_b(P):
                """den chain + coefficient maps (mostly DVE)."""
                sl = P.sl
                S1b, S2b = P.d12t[:, 0:PW], P.d12t[:, PW:2 * PW]
                S3b = P.d3q[:, 0:PW]
                p2b = spool.tile([128, 2 * PW], bf16, tag=f"p2b{sl}")
                t3 = spool.tile([128, PW], bf16, tag=f"t3{sl}")
                t4 = spool.tile([128, PW], bf16, tag=f"t4{sl}")
                # d12c parked in d3q's free upper half -> [S3c|d12c] pairs
                # with [oxy|oxx] for one fused 2048-wide den product
                d12c = P.d3q[:, PW:2 * PW]
                nc.vector.tensor_sub(out=d12c, in0=S1b, in1=S2b)
                nc.vector.tensor_mul(out=p2b[:], in0=P.oo[:], in1=P.d3q[:])
                nc.vector.tensor_add(out=t3[:], in0=p2b[:, 0:PW],
                                     in1=p2b[:, PW:2 * PW])
                nc.vector.tensor_add(out=t4[:], in0=t3[:], in1=S2b)
                e4 = opool.tile([128, PW], fp32, tag=f"of{sl}")
                nc.vector.tensor_scalar(out=e4[:], in0=t4[:], scalar1=4.0,
                                        scalar2=4e-6, op0=Alu.mult,
                                        op1=Alu.add)
                nc.vector.reciprocal_approx_fast(out=e4[:], in_=e4[:])
                qrb = p2b[:, PW:2 * PW]
                nc.scalar.copy(qrb, e4[:])
                AB = fpool.tile([128, 2 * PW], bf16, tag=f"AB{sl}p{P.wp}")
                C = fpool.tile([128, PW], bf16, tag=f"C{sl}p{P.wp}")
                nc.vector.tensor_mul(out=AB[:, 0:PW], in0=qrb, in1=P.oxx[:])
                nc.vector.tensor_mul(out=C[:], in0=qrb, in1=P.oxy[:])
                nc.vector.tensor_sub(out=AB[:, PW:2 * PW], in0=qrb,
                                     in1=AB[:, 0:PW])
                P.AB, P.C = AB, C

            def emit_produce(P, it):
                sl = P.sl
                cs = spool.tile([128, TILE_W], bf16, tag=f"cs{sl}")
                nc.vector.tensor_mul(out=cs[:], in0=P.c_t[:], in1=P.s_cur[:])
                P.cs = cs

            def emit_mm(P, it):
                sl = P.sl
                cs, s = P.cs, P.s_cur
                qA = qpool.tile([128, 2 * PW], fp32, tag="qA")
                qB = qpool.tile([128, 2 * PW], fp32, tag="qB")
                d12 = spool.tile([128, 2 * PW], bf16, tag=f"d13{sl}")
                d3q = spool.tile([128, 2 * PW], bf16, tag=f"d2q{sl}")
                emit_mm_group(qA[:, 0:PW], [
                    (tBm, lambda lo, hi: cs[:, W0 - 1 + lo:W0 - 1 + hi]),
                    (tBm, lambda lo, hi: cs[:, W0 + 1 + lo:W0 + 1 + hi]),
                ])
                emit_mm_group(qA[:, PW:2 * PW], [
                    (tT0, lambda lo, hi: cs[:, W0 + lo:W0 + hi]),
                    (tH0, lambda lo, hi: cs[:, W0 - 1 + lo:W0 - 1 + hi]),
                    (tH0, lambda lo, hi: cs[:, W0 + 1 + lo:W0 + 1 + hi]),
                ])
                nc.scalar.copy(d12[:], qA[:])
                emit_mm_group(qB[:, 0:PW], [
                    (tA0v, lambda lo, hi: cs[:, W0 + 1 + lo:W0 + 1 + hi]),
                    (tA0vm, lambda lo, hi: cs[:, W0 - 1 + lo:W0 - 1 + hi]),
                ])
                emit_mm_group(qB[:, PW:2 * PW], [
                    (tV36, lambda lo, hi: s[:, W0 - 1 + lo:W0 - 1 + hi]),
                    (tV36, lambda lo, hi: s[:, W0 + 1 + lo:W0 + 1 + hi]),
                    (tT8, lambda lo, hi: s[:, W0 + lo:W0 + hi]),
                ])
                nc.scalar.copy(d3q[:], qB[:])
                P.d12t, P.d3q = d12, d3q

            def emit_tail(P, it):
                sl = P.sl
                S3b, Qdb = P.d3q[:, 0:PW], P.d3q[:, PW:2 * PW]
                # fused [A|B] .* [S1|S2] in one 2048-wide 2x op
                p2b = spool.tile([128, 2 * PW], bf16, tag=f"p2b{sl}")
                t3 = spool.tile([128, PW], bf16, tag=f"t3{sl}")
                t4 = spool.tile([128, PW], bf16, tag=f"t4{sl}")
                nc.vector.tensor_mul(out=p2b[:], in0=P.AB[:], in1=P.d12t[:])
                nc.vector.tensor_add(out=t3[:], in0=p2b[:, 0:PW],
                                     in1=p2b[:, PW:2 * PW])
                nc.vector.tensor_mul(out=t4[:], in0=P.C[:], in1=S3b)
                nc.vector.tensor_add(out=t4[:], in0=t4[:], in1=Qdb)
                nc.vector.tensor_add(out=t3[:], in0=t3[:], in1=t4[:])
                pg = spool.tile([128, PW], bf16, tag=f"t4{sl}")
                nc.vector.tensor_mul(out=pg[:], in0=P.g[:], in1=t3[:])
                last = it == NUM_ITERS - 1
                if last:
                    yb = spool.tile([128, PW], bf16, tag=f"t3{sl}")
                    nc.vector.tensor_add(out=yb[:], in0=P.s_cur[:, WSL],
                                         in1=pg[:])
                    s_nxt = opool.tile([128, PW], fp32, tag=f"of{sl}")
                    nc.scalar.activation(s_nxt[:], yb[:], Act.Copy,
                                         bias=0.0, scale=0.75 ** NUM_ITERS)
                else:
                    s_nxt = s_ab[P.sl][it % 2]
                    nc.vector.tensor_add(out=s_nxt[:, WSL],
                                         in0=P.s_cur[:, WSL], in1=pg[:])
                P.s_cur = s_nxt

            def emit_store(P):
                nrows = P.r1 - P.r0
                nc.sync.dma_start(out=out_d[P.r0:P.r1, :],
                                  in_=P.s_cur[6:6 + nrows, :])

            waves = []
            for wi in range(0, len(row_panels), N_SLOTS):
                waves.append([make_panel(*row_panels[wi + j], j,
                                         (wi // N_SLOTS) % 2)
                              for j in range(min(N_SLOTS, len(row_panels) - wi))])
            stage_consts()
            tBm, tT0, tH0, tA0v, tA0vm, tV36, tT8 = st_b
            for P in waves[0]:
                emit_load(P)
            for w, wave in enumerate(waves):
                if w == 0:
                    # it=0 produce/mm depend only on loads, not on the
                    # coefficient pre-chain: start the PE early and let the
                    # den-chain overlap it.
                    for P in wave:
                        emit_produce(P, 0)
                    for P in wave:
                        emit_pre_a(P)
                    for P in wave:
                        emit_pre_b(P)
                    for P in wave:
                        emit_mm(P, 0)
                for it in range(NUM_ITERS):
                    if w == 0 and it == 0:
                        for P in wave:
                            emit_tail(P, it)
                        continue
                    for P in wave:
                        emit_produce(P, it)
                    for P in wave:
                        emit_mm(P, it)
                    for P in wave:
                        emit_tail(P, it)
                    # interleave next wave's loads + pre into this wave's
                    # iterations so engines stay fed across wave boundaries
                    if w + 1 < len(waves):
                        nxt = waves[w + 1]
                        if it == 1:
                            for P in nxt:
                                emit_load(P)
                        elif it == 3:
                            for P in nxt:
                                emit_pre_a(P)
                                emit_pre_b(P)
                for P in wave:
                    emit_store(P)

    nc.finalize()
    return nc


def kernel(center, continuity, orientation, uncertainty):
    from concourse.bass_utils import run_bass_kernel_spmd

    if "nc" not in _CACHE:
        _CACHE["nc"] = _build_bass()
    nc = _CACHE["nc"]

    import ml_dtypes
    bf = ml_dtypes.bfloat16
    B = center.shape[0]
    in_maps = []
    for b in range(B):
        in_maps.append({
            "center": np.ascontiguousarray(center[b, 0]).astype(bf),
            "continuity": np.ascontiguousarray(continuity[b, 0]).astype(bf),
            "orientation": np.ascontiguousarray(orientation[b]).astype(bf),
            "uncertainty": np.ascontiguousarray(uncertainty[b, 0]).astype(bf),
        })
    res = run_bass_kernel_spmd(nc, in_maps, core_ids=list(range(B)))
    out = np.stack([r["out"] for r in res.results])[:, None]
    return out.astype(np.float32)


# revision 44
# speedup vs baseline: 1.0284x; 1.0028x over previous
"""Trainium2 Bass kernel for nn_IterativeStructureRefiner (v5, bracket form).

Math (validated vs reference in numpy: fp32 2.9e-7, bf16-quantized 3.6e-3):
  s' = 0.75*s + g .* [ Qd + A.*S1 + B.*S2 + C.*S3 ]
    g  = 1 - unc                      (gate, applied once per iter)
    Qd = 0.25*(box3x3(s) - s)         (PE: V36@sL + V36@sR + T8@s)
    S1 = Bm@hx, S2 = T0@cs + H0@hx, S3 = A0v@csR - A0v@csL
         (cs = continuity*s, hx = csL+csR)
    A  = q*oxx, B = q - A, C = q*oxy,  q = 0.25/(den+eps)  (iteration-
    invariant; den = same stencils on continuity, using oyy = 1-oxx).

State substitution y_t = s_t/0.75^t makes the update y' = y + g'.*bracket
(g' = (1-unc)/0.75), so each iteration ends in a plain tensor_add; the
0.75^6 is folded into the final fp32 store-copy on the Scalar engine.

Engine split per strip-iteration (128x1024) - GpSimd is deliberately
IDLE: it shares an SBUF port with the Vector engine and 2-port DVE
tensor_tensor ops starve it (measured 2.4us ops inflating to 8-11us,
stalling the PE chain and oscillating HAM):
  Vector: cs, fused [A|B].*[S1|S2] (one 2048-wide 2x op), C-product,
          3 adds, gate product, y-add (8 tensor_tensor ops/iter; hx is
          absorbed into the PE via Bm/H0 passes on csL and csR)
  Scalar: 2 super-drains per iter (PSUM [S1|S2] and [S3|Qd] pairs as
          single 2048-wide fp32->bf16 ACTIVATE copies)
  Tensor: 20 matmuls (512-col fp32-PSUM chunks), 7 banded stationaries
  The pre-phase den product is similarly fused: [oxy|oxx].*[S3c|d12c]
  with d12c parked in the d3q super-tile's free upper half.

Geometry: 9 full-width row strips [128 x 1024], 6-row shrink halo (116
valid rows/strip). 3 strips in flight (one wave), 3 waves; next wave's
DMA loads (at it==1) and full pre-chain (at it==3) are interleaved into
the current wave's iterations so wave boundaries stay busy; wave 0 hoists
it=0 produce/matmuls ahead of the den-chain. PSUM: 2 shared [128,2048]
fp32 super-tiles (8 banks), drained promptly so slots rotate.

Sharding: pure data-parallel, one batch image per NeuronCore (B=8).
"""

import numpy as np

H = W = 1024
TILE_W = 1040          # tile col t <-> image col t - 8 ; pads [0:8) [1032:1040)
W0, W1 = 8, 1032
PW = W1 - W0           # 1024
ROWS_OUT = 116
NUM_ITERS = 6
N_SLOTS = 3

_CACHE = {}


def _build_bass():
    import concourse.bacc as bacc
    import concourse.mybir as mybir
    from concourse.tile import TileContext

    fp32 = mybir.dt.float32
    bf16 = mybir.dt.bfloat16
    Alu = mybir.AluOpType
    Act = mybir.ActivationFunctionType

    nc = bacc.Bacc("TRN2", debug=False)

    cen_d = nc.dram_tensor("center", [H, W], bf16, kind="ExternalInput")
    con_d = nc.dram_tensor("continuity", [H, W], bf16, kind="ExternalInput")
    ori_d = nc.dram_tensor("orientation", [2, H, W], bf16, kind="ExternalInput")
    unc_d = nc.dram_tensor("uncertainty", [H, W], bf16, kind="ExternalInput")
    out_d = nc.dram_tensor("out", [H, W], fp32, kind="ExternalOutput")

    # Stationaries (lhsT convention: out[i] = sum_k lhsT[k,i] * x[k]).
    k = np.arange(128)
    I128 = np.eye(128, dtype=np.float32)
    T0m_ = (np.abs(k[:, None] - k[None, :]) == 1).astype(np.float32)
    Bm = I128 + 0.5 * T0m_
    H0 = 0.5 * T0m_
    A0v = ((k[:, None] == k[None, :] + 1).astype(np.float32)
           - (k[:, None] == k[None, :] - 1).astype(np.float32))
    V36 = (I128 + T0m_) * (1.0 / 36.0)
    T8 = (T0m_ - 8.0 * I128) * (1.0 / 36.0)
    ST = [Bm, T0m_, H0, A0v, -A0v, V36, T8]
    st_drams = [nc.inline_tensor(m, name=f"st_{i}") for i, m in enumerate(ST)]

    row_panels = []
    for r0 in range(0, H, ROWS_OUT):
        row_panels.append((r0, min(r0 + ROWS_OUT, H)))
    assert len(row_panels) == 9

    CH = ((0, 512), (512, 1024))   # psum col j <-> tile col j + W0
    WSL = slice(W0, W1)

    with TileContext(nc) as tc:
        with (
            tc.tile_pool(name="consts", bufs=1) as kpool,
            tc.tile_pool(name="cin", bufs=2) as c2pool,
            tc.tile_pool(name="inp", bufs=2) as ipool,
            tc.tile_pool(name="coef", bufs=1) as fpool,
            tc.tile_pool(name="state", bufs=1) as spool,
            tc.tile_pool(name="outp", bufs=1) as opool,
            tc.tile_pool(name="psum", bufs=1, space="PSUM") as qpool,
        ):
            st_b = []
            s_ab = {}

            def stage_consts():
                for i, d in enumerate(st_drams):
                    tf = kpool.tile([128, 128], fp32, tag="stf")
                    nc.sync.dma_start(out=tf[:], in_=d[:, :])
                    tb = kpool.tile([128, 128], bf16, tag=f"st{i}")
                    nc.scalar.copy(tb[:], tf[:])
                    st_b.append(tb)
                # persistent s ping-pong tiles per slot; pads zeroed once
                for sl in range(N_SLOTS):
                    pair = []
                    for nm in ("sa", "sb"):
                        t = kpool.tile([128, TILE_W], bf16, tag=f"{nm}{sl}")
                        nc.vector.memset(t[:, 0:W0], 0.0)
                        nc.vector.memset(t[:, W1:TILE_W], 0.0)
                        pair.append(t)
                    s_ab[sl] = pair

            class Panel:
                pass

            def make_panel(r0, r1, sl, wp):
                P = Panel()
                P.r0, P.r1, P.sl, P.wp = r0, r1, sl, wp
                P.row_lo = max(r0 - 6, 0)
                P.row_hi = min(r0 + 122, H)
                P.p_lo = P.row_lo - (r0 - 6)
                P.p_hi = P.row_hi - (r0 - 6)
                return P

            def emit_load(P):
                sl = P.sl

                def row_pads(t, lo_val, hi_val, win):
                    if P.p_lo > 0:
                        nc.gpsimd.memset(t[0:P.p_lo, win], lo_val)
                    if P.p_hi < 128:
                        al = (P.p_hi // 32) * 32
                        nc.gpsimd.memset(t[al:128, win], hi_val)

                # continuity: padded 1040-wide tile (feeds stencils)
                ct = c2pool.tile([128, TILE_W], bf16, tag=f"c{sl}")
                nc.vector.memset(ct[:, 0:W0], 0.0)
                nc.vector.memset(ct[:, W1:TILE_W], 0.0)
                row_pads(ct, 0.0, 0.0, WSL)
                nc.sync.dma_start(out=ct[P.p_lo:P.p_hi, WSL],
                                  in_=con_d[P.row_lo:P.row_hi, :])
                P.c_t = ct
                # center: padded 1040-wide (initial s)
                st = ipool.tile([128, TILE_W], bf16, tag=f"s0{sl}")
                nc.vector.memset(st[:, 0:W0], 0.0)
                nc.vector.memset(st[:, W1:TILE_W], 0.0)
                row_pads(st, 0.0, 0.0, WSL)
                nc.sync.dma_start(out=st[P.p_lo:P.p_hi, WSL],
                                  in_=cen_d[P.row_lo:P.row_hi, :])
                P.s0_t = st
                P.s_cur = st
                # ox, oy: 1024-wide, zero row pads (NaN hygiene)
                full = slice(0, PW)
                for tag, src in ((f"ox{sl}", ori_d[0]), (f"oy{sl}", ori_d[1])):
                    t = ipool.tile([128, PW], bf16, tag=tag)
                    row_pads(t, 0.0, 0.0, full)
                    nc.sync.dma_start(out=t[P.p_lo:P.p_hi, :],
                                      in_=src[P.row_lo:P.row_hi, :])
                    setattr(P, "t_" + tag[:2], t)
                # uncertainty: pads = 1.0 so g = 1-unc = 0 outside image
                t = ipool.tile([128, PW], bf16, tag=f"un{sl}")
                row_pads(t, 1.0, 1.0, full)
                nc.sync.dma_start(out=t[P.p_lo:P.p_hi, :],
                                  in_=unc_d[P.row_lo:P.row_hi, :])
                P.t_un = t

            def emit_mm_group(q, passes):
                """passes = [(stationary, moving_window_fn)]; accumulate into
                PSUM region q (column range of a super-tile)."""
                n = len(passes)
                for pi, (stat, win) in enumerate(passes):
                    for lo, hi in CH:
                        nc.tensor.matmul(q[:, lo:hi], stat[:], win(lo, hi),
                                         start=(pi == 0), stop=(pi == n - 1))

            def emit_pre_a(P):
                """Maps + den stencils on PE + drains (cheap engines)."""
                sl = P.sl
                # paired [oxy|oxx] map tile enables a fused den-product below
                oo = spool.tile([128, 2 * PW], bf16, tag=f"oo{sl}")
                oxy, oxx = oo[:, 0:PW], oo[:, PW:2 * PW]
                g = fpool.tile([128, PW], bf16, tag=f"g{sl}p{P.wp}")
                nc.scalar.activation(oxx, P.t_ox[:], Act.Square)
                nc.vector.tensor_mul(out=oxy, in0=P.t_ox[:], in1=P.t_oy[:])
                # state substitution y_t = s_t / 0.75^t turns the update into
                # y' = y + g'.*bracket with g' = (1-unc)/0.75; the 0.75^6 is
                # applied once in the final fp32 store-copy.
                nc.scalar.activation(g[:], P.t_un[:], Act.Copy,
                                     bias=1.0 / 0.75, scale=-1.0 / 0.75)
                P.oo, P.oxx, P.oxy, P.g = oo, oxx, oxy, g

                c = P.c_t
                qA = qpool.tile([128, 2 * PW], fp32, tag="qA")
                qB = qpool.tile([128, 2 * PW], fp32, tag="qB")
                d12 = spool.tile([128, 2 * PW], bf16, tag=f"d13{sl}")
                d3q = spool.tile([128, 2 * PW], bf16, tag=f"d2q{sl}")
                emit_mm_group(qA[:, 0:PW], [
                    (tBm, lambda lo, hi: c[:, W0 - 1 + lo:W0 - 1 + hi]),
                    (tBm, lambda lo, hi: c[:, W0 + 1 + lo:W0 + 1 + hi]),
                ])
                emit_mm_group(qA[:, PW:2 * PW], [
                    (tT0, lambda lo, hi: c[:, W0 + lo:W0 + hi]),
                    (tH0, lambda lo, hi: c[:, W0 - 1 + lo:W0 - 1 + hi]),
                    (tH0, lambda lo, hi: c[:, W0 + 1 + lo:W0 + 1 + hi]),
                ])
                nc.scalar.copy(d12[:], qA[:])
                emit_mm_group(qB[:, 0:PW], [
                    (tA0v, lambda lo, hi: c[:, W0 + 1 + lo:W0 + 1 + hi]),
                    (tA0vm, lambda lo, hi: c[:, W0 - 1 + lo:W0 - 1 + hi]),
                ])
                nc.scalar.copy(d3q[:, 0:PW], qB[:, 0:PW])
                P.d12t, P.d3q = d12, d3q

            def emit_pre_b(P):
                """den chain + coefficient maps (mostly DVE)."""
                sl = P.sl
                S1b, S2b = P.d12t[:, 0:PW], P.d12t[:, PW:2 * PW]
                S3b = P.d3q[:, 0:PW]
                p2b = spool.tile([128, 2 * PW], bf16, tag=f"p2b{sl}")
                t3 = spool.tile([128, PW], bf16, tag=f"t3{sl}")
                t4 = spool.tile([128, PW], bf16, tag=f"t4{sl}")
                # d12c parked in d3q's free upper half -> [S3c|d12c] pairs
                # with [oxy|oxx] for one fused 2048-wide den product
                d12c = P.d3q[:, PW:2 * PW]
                nc.vector.tensor_sub(out=d12c, in0=S1b, in1=S2b)
                nc.vector.tensor_mul(out=p2b[:], in0=P.oo[:], in1=P.d3q[:])
                nc.vector.tensor_add(out=t3[:], in0=p2b[:, 0:PW],
                                     in1=p2b[:, PW:2 * PW])
                nc.vector.tensor_add(out=t4[:], in0=t3[:], in1=S2b)
                e4 = opool.tile([128, PW], fp32, tag=f"of{sl}")
                nc.vector.tensor_scalar(out=e4[:], in0=t4[:], scalar1=4.0,
                                        scalar2=4e-6, op0=Alu.mult,
                                        op1=Alu.add)
                nc.vector.reciprocal_approx_fast(out=e4[:], in_=e4[:])
                qrb = p2b[:, PW:2 * PW]
                nc.scalar.copy(qrb, e4[:])
                AB = fpool.tile([128, 2 * PW], bf16, tag=f"AB{sl}p{P.wp}")
                C = fpool.tile([128, PW], bf16, tag=f"C{sl}p{P.wp}")
                nc.vector.tensor_mul(out=AB[:, 0:PW], in0=qrb, in1=P.oxx[:])
                nc.vector.tensor_mul(out=C[:], in0=qrb, in1=P.oxy[:])
                nc.vector.tensor_sub(out=AB[:, PW:2 * PW], in0=qrb,
                                     in1=AB[:, 0:PW])
                P.AB, P.C = AB, C

            def emit_produce(P, it):
                sl = P.sl
                cs = spool.tile([128, TILE_W], bf16, tag=f"cs{sl}")
                nc.vector.tensor_mul(out=cs[:], in0=P.c_t[:], in1=P.s_cur[:])
                P.cs = cs

            def emit_mm(P, it):
                sl = P.sl
                cs, s = P.cs, P.s_cur
                qA = qpool.tile([128, 2 * PW], fp32, tag="qA")
                qB = qpool.tile([128, 2 * PW], fp32, tag="qB")
                d12 = spool.tile([128, 2 * PW], bf16, tag=f"d13{sl}")
                d3q = spool.tile([128, 2 * PW], bf16, tag=f"d2q{sl}")
                emit_mm_group(qA[:, 0:PW], [
                    (tBm, lambda lo, hi: cs[:, W0 - 1 + lo:W0 - 1 + hi]),
                    (tBm, lambda lo, hi: cs[:, W0 + 1 + lo:W0 + 1 + hi]),
                ])
                emit_mm_group(qA[:, PW:2 * PW], [
                    (tT0, lambda lo, hi: cs[:, W0 + lo:W0 + hi]),
                    (tH0, lambda lo, hi: cs[:, W0 - 1 + lo:W0 - 1 + hi]),
                    (tH0, lambda lo, hi: cs[:, W0 + 1 + lo:W0 + 1 + hi]),
                ])
                nc.scalar.copy(d12[:], qA[:])
                emit_mm_group(qB[:, 0:PW], [
                    (tA0v, lambda lo, hi: cs[:, W0 + 1 + lo:W0 + 1 + hi]),
                    (tA0vm, lambda lo, hi: cs[:, W0 - 1 + lo:W0 - 1 + hi]),
                ])
                emit_mm_group(qB[:, PW:2 * PW], [
                    (tV36, lambda lo, hi: s[:, W0 - 1 + lo:W0 - 1 + hi]),
                    (tV36, lambda lo, hi: s[:, W0 + 1 + lo:W0 + 1 + hi]),
                    (tT8, lambda lo, hi: s[:, W0 + lo:W0 + hi]),
                ])
                nc.scalar.copy(d3q[:], qB[:])
                P.d12t, P.d3q = d12, d3q

            def emit_tail(P, it):
                sl = P.sl
                S3b, Qdb = P.d3q[:, 0:PW], P.d3q[:, PW:2 * PW]
                # fused [A|B] .* [S1|S2] in one 2048-wide 2x op
                p2b = spool.tile([128, 2 * PW], bf16, tag=f"p2b{sl}")
                t3 = spool.tile([128, PW], bf16, tag=f"t3{sl}")
                t4 = spool.tile([128, PW], bf16, tag=f"t4{sl}")
                nc.vector.tensor_mul(out=p2b[:], in0=P.AB[:], in1=P.d12t[:])
                nc.vector.tensor_add(out=t3[:], in0=p2b[:, 0:PW],
                                     in1=p2b[:, PW:2 * PW])
                nc.vector.tensor_mul(out=t4[:], in0=P.C[:], in1=S3b)
                nc.vector.tensor_add(out=t4[:], in0=t4[:], in1=Qdb)
                nc.vector.tensor_add(out=t3[:], in0=t3[:], in1=t4[:])
                pg = spool.tile([128, PW], bf16, tag=f"t4{sl}")
                nc.vector.tensor_mul(out=pg[:], in0=P.g[:], in1=t3[:])
                last = it == NUM_ITERS - 1
                if last:
                    yb = spool.tile([128, PW], bf16, tag=f"t3{sl}")
                    nc.vector.tensor_add(out=yb[:], in0=P.s_cur[:, WSL],
                                         in1=pg[:])
                    s_nxt = opool.tile([128, PW], fp32, tag=f"of{sl}")
                    nc.scalar.activation(s_nxt[:], yb[:], Act.Copy,
                                         bias=0.0, scale=0.75 ** NUM_ITERS)
                else:
                    s_nxt = s_ab[P.sl][it % 2]
                    nc.vector.tensor_add(out=s_nxt[:, WSL],
                                         in0=P.s_cur[:, WSL], in1=pg[:])
                P.s_cur = s_nxt

            def emit_store(P):
                nrows = P.r1 - P.r0
                nc.sync.dma_start(out=out_d[P.r0:P.r1, :],
                                  in_=P.s_cur[6:6 + nrows, :])

            waves = []
            for wi in range(0, len(row_panels), N_SLOTS):
                waves.append([make_panel(*row_panels[wi + j], j,
                                         (wi // N_SLOTS) % 2)
                              for j in range(min(N_SLOTS, len(row_panels) - wi))])
            stage_consts()
            tBm, tT0, tH0, tA0v, tA0vm, tV36, tT8 = st_b
            for P in waves[0]:
                emit_load(P)
            for w, wave in enumerate(waves):
                if w == 0:
                    # it=0 produce/mm depend only on loads, not on the
                    # coefficient pre-chain: start the PE early and let the
                    # den-chain overlap it.
                    for P in wave:
                        emit_produce(P, 0)
                    for P in wave:
                        emit_pre_a(P)
                    for P in wave:
                        emit_pre_b(P)
                    for P in wave:
                        emit_mm(P, 0)
                for it in range(NUM_ITERS):
                    if it == 0:
                        # it0 produce/mm were pre-emitted (wave 0: above;
                        # later waves: at the previous wave's it5) so the PE
                        # stream runs through the wave boundary
                        for P in wave:
                            emit_tail(P, it)
                        continue
                    for P in wave:
                        emit_produce(P, it)
                    for P in wave:
                        emit_mm(P, it)
                    last_it = it == NUM_ITERS - 1
                    if last_it and w + 1 < len(waves):
                        for P in waves[w + 1]:
                            emit_produce(P, 0)
                    for P in wave:
                        emit_tail(P, it)
                    if last_it and w + 1 < len(waves):
                        for P in waves[w + 1]:
                            emit_mm(P, 0)
                    # interleave next wave's loads + pre into this wave's
                    # iterations so engines stay fed across wave boundaries
                    if w + 1 < len(waves):
                        nxt = waves[w + 1]
                        if it == 1:
                            for P in nxt:
                                emit_load(P)
                        elif it == 3:
                            for P in nxt:
                                emit_pre_a(P)
                                emit_pre_b(P)
                for P in wave:
                    emit_store(P)

    nc.finalize()
    return nc


def kernel(center, continuity, orientation, uncertainty):
    from concourse.bass_utils import run_bass_kernel_spmd

    if "nc" not in _CACHE:
        _CACHE["nc"] = _build_bass()
    nc = _CACHE["nc"]

    import ml_dtypes
    bf = ml_dtypes.bfloat16
    B = center.shape[0]
    in_maps = []
    for b in range(B):
        in_maps.append({
            "center": np.ascontiguousarray(center[b, 0]).astype(bf),
            "continuity": np.ascontiguousarray(continuity[b, 0]).astype(bf),
            "orientation": np.ascontiguousarray(orientation[b]).astype(bf),
            "uncertainty": np.ascontiguousarray(uncertainty[b, 0]).astype(bf),
        })
    res = run_bass_kernel_spmd(nc, in_maps, core_ids=list(range(B)))
    out = np.stack([r["out"] for r in res.results])[:, None]
    return out.astype(np.float32)
